# revision 1
# baseline (speedup 1.0000x reference)
"""Trainium2 Bass kernel for nn_CrossConvLayerV2 (gnn_message_passing).

Math (reference):
    coords = points[..., :3]; feats = points[..., 3:]          # [B,n,3], [B,n,f]
    probes[b,l,m] = centers[b,l] + PROBES[m]                    # [B,l,m,3]
    sq[b,l,m,n]  = ||coords[b,n] - probes[b,l,m]||^2
    kern         = C / (sq + C)          (C = 0.1)
    agg[b,l,m,f] = (1/n) sum_n kern * feats
    out[b,l,:]   = agg.reshape(l, m*f) @ W + bias               # [B,l,256]

Strategy:
  - Shard centers dim l (256) over 8 cores -> 32 centers/core, zero
    communication; the host gathers the 8 [B,32,256] shards.
  - The [B,l,m,n] kernel tensor never touches HBM. Per 128-point chunk t
    and per job (b, 16-center slab):
      u = 10*sq + 1 via ONE K=24 bf16 matmul of "expanded" vectors.
        The squared distance is expanded as cn2 - 2<c,p> + pn2; every
        quantity is split into THREE bf16 parts (hi/mid/lo, 24 mantissa
        bits total). bf16 x bf16 products are EXACT on the PE (measured)
        and accumulate in fp32, so u is accurate to ~2^-24 relative —
        the contraction depth K is free (cost is moving-dim bound).
      kern = 1/u on the ACT engine (Reciprocal LUT, measured 1.2e-5 rel;
        optionally some chunks on DVE's exact-but-slow reciprocal),
        written as fp16 (11-bit, exact products downstream).
      agg[f, (m,l')] += feats_chunk^T @ kern^T   (fp16, PSUM accumulate)
      out[l', :] = sum_m aggS[:, m-slice]^T @ W_m  (fp16 matmuls)
  - b_weighter is added on the host (it is zeros for this problem); the
    1/n mean is folded into W on the host.
  - This walrus build encodes at most ONE semaphore wait per instruction;
    a post-build pass splits multi-wait instructions into single-wait
    NoOp carriers.
"""

import sys

sys.path.insert(0, "/opt/trn_rl_repo")

import numpy as np
import ml_dtypes

# ---- problem constants (hardcoded per contract) ----
B, N, L, D, F = 2, 4096, 256, 3, 16
M = 26
OUT_D = 256
COEFF = 0.1
DIST = 3.0
N_CORES = 8
L_LOC = L // N_CORES          # 32 centers per core
N_SLABS = 2                   # jobs per batch elem per core
L_SLAB = L_LOC // N_SLABS     # 16 centers per job
JM = M * L_SLAB               # 416 = free dim of kern^T tiles
N_JOBS = B * N_SLABS          # 4 jobs per core
NT = N // 128                 # 32 n-chunks
K5 = 24                       # expanded-distance contraction depth

# how many of every 8 n-chunks run the reciprocal on DVE (rest on ACT)
RECIP_DVE_OF8 = 0


def _make_probes() -> np.ndarray:
    angles = np.array(
        [[j * 0.125 - 0.125, i * 0.125 + (j - 1) * 0.0625] for j in range(3) for i in range(8)]
        + [[-0.25, 0.0], [0.25, 0.0]],
        dtype=np.float64,
    ) * (2.0 * np.pi)
    a, b = angles[:, 0], angles[:, 1]
    pts = np.stack([np.sin(a), np.cos(a) * np.cos(b), np.cos(a) * np.sin(b)], axis=-1) * DIST
    return pts.astype(np.float32)  # [26, 3]


PROBES = _make_probes()


def _split3_bf16(x):
    """x (f64) -> three bf16 arrays whose sum approximates x to ~24 bits."""
    x0 = x.astype(ml_dtypes.bfloat16)
    r1 = x - x0.astype(np.float64)
    x1 = r1.astype(ml_dtypes.bfloat16)
    x2 = (r1 - x1.astype(np.float64)).astype(ml_dtypes.bfloat16)
    return x0, x1, x2


_NC = None
_NC_KEY = None


def _act_reciprocal(nc, out_ap, in_ap):
    """nc.scalar.activation(func=Reciprocal) minus the library guard.
    out = 1/in_ on the ACT engine (LUT path; measured ~1.2e-5 rel here)."""
    import concourse.mybir as mybir

    eng = nc.scalar
    inputs = [eng.lower_ap(in_ap)]
    for val in (0.0, 1.0, 0.0):  # bias, scale, alpha — immediates
        inputs.append(mybir.ImmediateValue(dtype=mybir.dt.float32, value=val))
    return eng.add_instruction(
        mybir.InstActivation(
            name=nc.get_next_instruction_name(),
            func=mybir.ActivationFunctionType.Reciprocal,
            ins=inputs,
            outs=[eng.lower_ap(out_ap)],
        )
    )


def _split_multi_waits(nc):
    """This walrus build encodes at most ONE semaphore wait per instruction.
    Split every instruction with k>1 waits into (k-1) single-wait NoOps on
    the same engine immediately before it — identical blocking semantics."""
    import concourse.mybir as mybir

    n = 0
    for f in nc.m.functions:
        for bb in f.blocks:
            new_il = []
            for inst in bb.instructions:
                si = inst.sync_info
                waits = list(si.on_wait) if si is not None else []
                if len(waits) > 1:
                    for w in waits[:-1]:
                        nop = mybir.InstNoOp(name=f"{inst.name}-wsplit{n}", ins=[], outs=[])
                        n += 1
                        nop.engine = inst.engine
                        nop.sync_info = mybir.SyncInfo(on_wait=[w], on_update=[])
                        nc.register_instruction(nop, overwrite=True)
                        new_il.append(nop)
                    inst.sync_info = mybir.SyncInfo(
                        on_wait=[waits[-1]], on_update=list(si.on_update)
                    )
                new_il.append(inst)
            bb.instructions = new_il
    return n


def _build_nc(recip_dve_of8=RECIP_DVE_OF8):
    import concourse.bass as bass
    import concourse.mybir as mybir
    import concourse.tile as tile

    f32 = mybir.dt.float32
    bf16 = mybir.dt.bfloat16
    fp16 = mybir.dt.float16

    nc = bass.Bass()
    c5_d = nc.dram_tensor("c5", [K5, B * N], bf16, kind="ExternalInput")
    p5_d = nc.dram_tensor("p5", [K5, N_JOBS * JM], bf16, kind="ExternalInput")
    ft_d = nc.dram_tensor("ft", [128, B * NT * F], fp16, kind="ExternalInput")
    # W*(8/n) in two bf16 pieces (hi, lo) — bf16 has fp32 exponent range,
    # so no subnormal flush risk at any scale; 2 pieces ~ 16 mantissa bits.
    wt_d = nc.dram_tensor("wt", [F, 2 * M * OUT_D], bf16, kind="ExternalInput")
    out_d = nc.dram_tensor("out", [N_JOBS * L_SLAB, OUT_D], f32, kind="ExternalOutput")

    with (
        nc.allow_low_precision(reason="split-bf16 matmul is ~24-bit exact"),
        tile.TileContext(nc) as tc,
    ):
        with (
            tc.tile_pool(name="const", bufs=1) as cpool,
            tc.tile_pool(name="kt", bufs=4) as ktpool,
            tc.tile_pool(name="sb", bufs=2) as sbpool,
            tc.tile_pool(name="sq", bufs=3, space="PSUM") as sqpool,
            tc.tile_pool(name="acc", bufs=2, space="PSUM") as accpool,
            tc.tile_pool(name="op", bufs=2, space="PSUM") as oppool,
        ):
            c5s = cpool.tile([K5, B * N], bf16)
            nc.sync.dma_start(c5s[:], c5_d[:, :])
            p5s = cpool.tile([K5, N_JOBS * JM], bf16)
            nc.sync.dma_start(p5s[:], p5_d[:, :])
            fts = cpool.tile([128, B * NT * F], fp16)
            nc.sync.dma_start(fts[:], ft_d[:, :])
            wts = cpool.tile([F, 2 * M * OUT_D], bf16)
            nc.sync.dma_start(wts[:], wt_d[:, :])
            agg0 = cpool.tile([F, N_JOBS * JM], bf16)
            agg1 = cpool.tile([F, N_JOBS * JM], bf16)

            from concourse.alu_op_type import AluOpType

            for jj in range(N_JOBS):
                b = jj // N_SLABS
                agg = accpool.tile([F, JM], f32, tag="agg")
                for t in range(NT):
                    sq = sqpool.tile([128, JM], f32, tag="sq")
                    nc.tensor.matmul(
                        sq[:],
                        lhsT=c5s[:, b * N + t * 128 : b * N + (t + 1) * 128],
                        rhs=p5s[:, jj * JM : (jj + 1) * JM],
                        start=True,
                        stop=True,
                    )
                    kt = ktpool.tile([128, JM], fp16, tag="kt")
                    if t % 8 < recip_dve_of8:
                        nc.vector.reciprocal(kt[:], sq[:])
                    else:
                        _act_reciprocal(nc, kt[:], sq[:])
                    nc.tensor.matmul(
                        agg[:],
                        lhsT=fts[:, (b * NT + t) * F : (b * NT + t + 1) * F],
                        rhs=kt[:],
                        start=(t == 0),
                        stop=(t == NT - 1),
                    )

                # split agg (f32 psum) into bf16 hi+lo pieces, stored m-major:
                # aggX free index = (m, jj, l') so each mi-slice is contiguous
                a0 = agg0[:].rearrange("p (m j l) -> p m j l", m=M, j=N_JOBS)[:, :, jj, :]
                a1 = agg1[:].rearrange("p (m j l) -> p m j l", m=M, j=N_JOBS)[:, :, jj, :]
                aggv = agg[:].rearrange("p (m l) -> p m l", m=M)
                nc.vector.tensor_copy(a0, aggv)
                nc.vector.tensor_tensor(a1, aggv, a0, AluOpType.subtract)

            # weighter, batched over all jobs: out[(jj,l'), o], 3 bf16 passes
            JL = N_JOBS * L_SLAB
            op = oppool.tile([JL, OUT_D], f32)
            first = True
            for ai, wi in ((0, 0), (0, 1), (1, 0)):
                asrc = (agg0, agg1)[ai]
                for mi in range(M):
                    nc.tensor.matmul(
                        op[:],
                        lhsT=asrc[:, mi * JL : (mi + 1) * JL],
                        rhs=wts[:, (wi * M + mi) * OUT_D : (wi * M + mi + 1) * OUT_D],
                        start=first,
                        stop=(ai, wi, mi) == (1, 0, M - 1),
                    )
                    first = False
            oS = sbpool.tile([JL, OUT_D], f32)
            nc.vector.tensor_copy(oS[:], op[:])
            nc.sync.dma_start(out_d[:, :], oS[:])

    _split_multi_waits(nc)
    return nc


def _get_nc(recip_dve_of8=RECIP_DVE_OF8):
    global _NC, _NC_KEY
    if _NC is None or _NC_KEY != recip_dve_of8:
        _NC = _build_nc(recip_dve_of8)
        _NC_KEY = recip_dve_of8
    return _NC


def _prep_shared(points, W_weighter):
    coords = points[:, :, :D].astype(np.float64)           # [B, n, 3]
    feats = points[:, :, D:].astype(np.float32)            # [B, n, f]
    q = 10.0 * (coords**2).sum(-1)                         # [B, n] f64

    # c5 rows (bf16): per coordinate k the six cross rows pair as
    #   [c0, c0, c1, c1, c2, c0] x [p0, p1, p0, p1, p0, p2]
    # then [1,1,1] x [r0,r1,r2] and [q0,q1,q2] x [1,1,1].
    c5 = np.zeros((K5, B * N), ml_dtypes.bfloat16)
    for b in range(B):
        s = slice(b * N, (b + 1) * N)
        for k in range(D):
            c0, c1, c2 = _split3_bf16(coords[b, :, k])
            base = 6 * k
            c5[base + 0, s] = c0
            c5[base + 1, s] = c0
            c5[base + 2, s] = c1
            c5[base + 3, s] = c1
            c5[base + 4, s] = c2
            c5[base + 5, s] = c0
        c5[18:21, s] = 1.0
        q0, q1, q2 = _split3_bf16(q[b])
        c5[21, s] = q0
        c5[22, s] = q1
        c5[23, s] = q2

    # ft[p, (b, t, f)] = feats[b, t*128+p, f]   (fp16)
    ft = (
        np.ascontiguousarray(feats.reshape(B, NT, 128, F).transpose(2, 0, 1, 3))
        .reshape(128, B * NT * F)
        .astype(np.float16)
    )

    # wt[f, (piece, m, o)] = piece_{0,1} of W[(m*F+f), o] * (8/n) in bf16.
    # (u is scaled by 8 on the probe side so fp16 kern=1/(8u') stays normal.)
    w8 = (
        np.ascontiguousarray(
            (W_weighter.astype(np.float64) * (8.0 / N)).reshape(M, F, OUT_D).transpose(1, 0, 2)
        ).reshape(F, M * OUT_D)
    )
    w0 = w8.astype(ml_dtypes.bfloat16)
    w1 = (w8 - w0.astype(np.float64)).astype(ml_dtypes.bfloat16)
    wt = np.concatenate([w0, w1], axis=1)  # [F, 2*M*OUT_D]
    return c5, ft, wt


def _prep_probes5(centers, core):
    cen = centers[:, core * L_LOC : (core + 1) * L_LOC, :].astype(np.float64)  # [B, 32, 3]
    p5 = np.zeros((K5, N_JOBS * JM), ml_dtypes.bfloat16)
    for b in range(B):
        for sl_i in range(N_SLABS):
            jj = b * N_SLABS + sl_i
            s = slice(jj * JM, (jj + 1) * JM)
            sl = cen[b, sl_i * L_SLAB : (sl_i + 1) * L_SLAB]       # [16, 3]
            pf = sl[:, None, :] + PROBES[None].astype(np.float64)  # [16, 26, 3]
            mlf = pf.transpose(1, 0, 2).reshape(JM, 3)             # (m, l') major
            for k in range(D):
                p0, p1, p2 = _split3_bf16(8.0 * -20.0 * mlf[:, k])
                base = 6 * k
                p5[base + 0, s] = p0
                p5[base + 1, s] = p1
                p5[base + 2, s] = p0
                p5[base + 3, s] = p1
                p5[base + 4, s] = p0
                p5[base + 5, s] = p2
            r = 8.0 * (10.0 * (mlf**2).sum(-1) + 1.0)              # [JM] f64
            r0, r1, r2 = _split3_bf16(r)
            p5[18, s] = r0
            p5[19, s] = r1
            p5[20, s] = r2
            p5[21:24, s] = 8.0
    return p5


def kernel(points, centers, W_weighter, b_weighter):
    from concourse.bass_utils import run_bass_kernel_spmd

    points = np.asarray(points)
    centers = np.asarray(centers)
    W_weighter = np.asarray(W_weighter)
    b_weighter = np.asarray(b_weighter)

    nc = _get_nc()
    c5, ft, wt = _prep_shared(points, W_weighter)
    in_maps = [
        {"c5": c5, "ft": ft, "p5": _prep_probes5(centers, core), "wt": wt}
        for core in range(N_CORES)
    ]
    res = run_bass_kernel_spmd(nc, in_maps, core_ids=list(range(N_CORES))).results

    out = np.empty((B, L, OUT_D), np.float32)
    for core in range(N_CORES):
        r = res[core]["out"]  # [(jj, l'), OUT_D]
        for jj in range(N_JOBS):
            b, s = jj // N_SLABS, jj % N_SLABS
            lo = core * L_LOC + s * L_SLAB
            out[b, lo : lo + L_SLAB] = r[jj * L_SLAB : (jj + 1) * L_SLAB]
    out += b_weighter.astype(np.float32)[None, None, :]
    return out

